# revision 1
# baseline (speedup 1.0000x reference)
"""Two-layer GCN (GCNConv 256->128->64, leaky_relu) on 8 Trainium2 NeuronCores.

Strategy (graph/data parallel per the sharding hint):
  - Nodes are sharded contiguously across the 8 cores (12500 each); each core
    owns the destination rows of the scatter for its node range.
  - Symmetric normalization is folded into node features: with
    h~ = dinv ! (X @ W), the aggregation for node v is
    out[v] = dinv[v] * (sum_{e: dst=v} h~[src_e] + h~[v]) + b.
  - Each layer: dense transform on local nodes -> AllGather h~ (the halo
    exchange; with 8 cores and uniform random edges every core needs nearly
    every node) -> per 128-node dst block, dma_gather the source rows and
    segment-sum them with 0/1 selection-matrix matmuls accumulated in PSUM.
  - Edge tables (gather indices, dst slots) are precomputed host-side from
    edge_index; edges are sorted by (dst block group, src 32k-chunk, dst
    block) and padded to 128-edge tiles.  dma_gather indices are int16, which
    limits each gather to a 32768-row window of the source table - hence the
    4 source chunks.
"""
import math
import sys

import numpy as np

sys.path.insert(0, "/opt/trn_rl_repo")

import concourse.bacc as bacc  # noqa: E402
import concourse.mybir as mybir  # noqa: E402
import concourse.tile as tile  # noqa: E402
from concourse.bass_utils import run_bass_kernel_spmd  # noqa: E402

P = 128
F32 = mybir.dt.float32
I16 = mybir.dt.int16


class Cfg:
    def __init__(self, n, e, d0, d1, d2, ncores=8, grp=6, chunk=32768,
                 neg_slope=0.01):
        assert n % ncores == 0
        self.n, self.e = n, e
        self.d0, self.d1, self.d2 = d0, d1, d2
        self.ncores = ncores
        self.nloc = n // ncores
        self.nblk = math.ceil(self.nloc / P)
        self.grp = grp
        self.ngrp = math.ceil(self.nblk / grp)
        self.chunk = chunk
        self.nchunk = math.ceil(n / chunk)
        self.neg_slope = neg_slope

    def rows(self, b):
        return min(P, self.nloc - b * P)


FULL = Cfg(n=100000, e=1600000, d0=256, d1=128, d2=64)


# --------------------------------------------------------------------------
# host-side preprocessing
# --------------------------------------------------------------------------

def prepare(cfg, x, edge_index, W1, b1, W2, b2):
    """Shard + build per-core tables and the shared tile structure."""
    src = np.asarray(edge_index[0], dtype=np.int64)
    dst = np.asarray(edge_index[1], dtype=np.int64)

    deg = (np.bincount(dst, minlength=cfg.n) + 1).astype(np.float32)
    dinv = (np.float32(1.0) / np.sqrt(deg)).astype(np.float32)

    core = dst // cfg.nloc
    dstl = dst - core * cfg.nloc
    b = dstl >> 7
    k = src // cfg.chunk
    g = b // cfg.grp
    # stream order: core -> g -> k -> b (tiles within segment arbitrary)
    key = ((core * cfg.ngrp + g) * cfg.nchunk + k) * cfg.nblk + b
    order = np.argsort(key, kind="stable")
    src_s, b_s, k_s, core_s, dstl_s = (
        src[order], b[order], k[order], core[order], dstl[order])

    seg_id = ((core_s * cfg.nblk + b_s) * cfg.nchunk + k_s)
    counts = np.bincount(seg_id, minlength=cfg.ncores * cfg.nblk * cfg.nchunk)
    counts = counts.reshape(cfg.ncores, cfg.nblk, cfg.nchunk)
    # shared tile structure: max over cores
    T = np.ceil(counts.max(axis=0) / P).astype(np.int64)  # [nblk, nchunk]

    # segment stream offsets (in padded edge slots), shared across cores
    seg_off = np.zeros((cfg.nblk, cfg.nchunk), np.int64)
    blocks_of_g = [list(range(gg * cfg.grp, min((gg + 1) * cfg.grp, cfg.nblk)))
                   for gg in range(cfg.ngrp)]
    tot_tiles = 0
    for gg in range(cfg.ngrp):
        for kk in range(cfg.nchunk):
            for bb in blocks_of_g[gg]:
                seg_off[bb, kk] = tot_tiles * P
                tot_tiles += int(T[bb, kk])
    etot = tot_tiles * P

    # per-core padded arrays
    xs = np.asarray(x, dtype=np.float32)
    W1 = np.asarray(W1, np.float32)
    W2 = np.asarray(W2, np.float32)
    b1 = np.asarray(b1, np.float32)
    b2 = np.asarray(b2, np.float32)
    b1bc = np.tile(b1[None, :], (P, 1)).astype(np.float32)
    b2bc = np.tile(b2[None, :], (P, 1)).astype(np.float32)

    in_maps = []
    for c in range(cfg.ncores):
        m = core_s == c
        src_c, b_c, k_c, dstl_c = src_s[m], b_s[m], k_s[m], dstl_s[m]
        seg_c = (b_c * cfg.nchunk + k_c)
        # rank within segment: edges are segment-contiguous in stream order
        ne = len(src_c)
        if ne:
            newseg = np.r_[True, seg_c[1:] != seg_c[:-1]]
            seg_start = np.maximum.accumulate(
                np.where(newseg, np.arange(ne), 0))
            pos_in_seg = np.arange(ne) - seg_start
        else:
            pos_in_seg = np.zeros(0, np.int64)
        padded_pos = seg_off[b_c, k_c] + pos_in_seg

        idx_pad = np.zeros(etot, np.int16)
        idx_pad[padded_pos] = (src_c - k_c * cfg.chunk).astype(np.int16)
        dr_pad = np.full(etot, -1.0, np.float32)
        dr_pad[padded_pos] = (dstl_c - b_c * P).astype(np.float32)

        # idx table: wrapped [16, etot//16] (entry [i%16, i//16]), x8 down 128
        idx16 = idx_pad.reshape(etot // 16, 16).T.copy()
        idx_tab = np.tile(idx16, (8, 1))
        # dstrel table: [128, tot_tiles], [p, t] = edge t*128+p
        dstrel_tab = dr_pad.reshape(tot_tiles, P).T.copy()

        lo = c * cfg.nloc
        xT = np.ascontiguousarray(xs[lo:lo + cfg.nloc].T)
        dloc = dinv[lo:lo + cfg.nloc]
        dcols = np.ones(cfg.nblk * P, np.float32)
        dcols[:cfg.nloc] = dloc
        dinv_cols = dcols.reshape(cfg.nblk, P).T.copy()

        in_maps.append({
            "xT": xT, "dinv_cols": dinv_cols,
            "W1": W1, "W2": W2, "b1bc": b1bc, "b2bc": b2bc,
            "idx_tab": idx_tab, "dstrel_tab": dstrel_tab,
        })

    struct = dict(T=T, blocks_of_g=blocks_of_g, tot_tiles=tot_tiles)
    return in_maps, struct


# --------------------------------------------------------------------------
# device program
# --------------------------------------------------------------------------

def build_program(cfg, struct, repeat=1):
    T = struct["T"]
    blocks_of_g = struct["blocks_of_g"]
    tot_tiles = struct["tot_tiles"]
    NB, NK, NG = cfg.nblk, cfg.nchunk, cfg.ngrp
    D0, D1, D2 = cfg.d0, cfg.d1, cfg.d2
    K0 = D0 // P
    G = cfg.grp

    # per-group layout in the tile stream
    grp_tiles = []           # tiles per group
    gk_info = []             # [g][k] -> (col0 within group, ntiles)
    blk_tile_cols = {}       # b -> [(col within group, ntiles, k)]
    grp_col0 = []            # group offset in global tile stream
    tot = 0
    for g in range(NG):
        grp_col0.append(tot)
        col = 0
        ks = []
        for k in range(NK):
            c0 = col
            for b in blocks_of_g[g]:
                t = int(T[b, k])
                if t:
                    blk_tile_cols.setdefault(b, []).append((col, t, k))
                    col += t
            ks.append((c0, col - c0))
        gk_info.append(ks)
        grp_tiles.append(col)
        tot += col
    assert tot == tot_tiles
    TG = max(grp_tiles)
    TBK = max(int(T.max()), 1)

    nc = bacc.Bacc("TRN2", target_bir_lowering=False, debug=False,
                   num_devices=cfg.ncores)
    xT = nc.dram_tensor("xT", [D0, cfg.nloc], F32, kind="ExternalInput")
    dinv_t = nc.dram_tensor("dinv_cols", [P, NB], F32, kind="ExternalInput")
    W1 = nc.dram_tensor("W1", [D0, D1], F32, kind="ExternalInput")
    W2 = nc.dram_tensor("W2", [D1, D2], F32, kind="ExternalInput")
    b1t = nc.dram_tensor("b1bc", [P, D1], F32, kind="ExternalInput")
    b2t = nc.dram_tensor("b2bc", [P, D2], F32, kind="ExternalInput")
    idx_t = nc.dram_tensor("idx_tab", [P, tot_tiles * 8], I16,
                           kind="ExternalInput")
    drel_t = nc.dram_tensor("dstrel_tab", [P, tot_tiles], F32,
                            kind="ExternalInput")
    out_t = nc.dram_tensor("out_loc", [cfg.nloc, D2], F32,
                           kind="ExternalOutput")

    eq = mybir.AluOpType.is_equal
    mul = mybir.AluOpType.mult
    add = mybir.AluOpType.add
    lrelu = mybir.ActivationFunctionType.Lrelu

    with tile.TileContext(nc) as tc:
        with (
            tc.tile_pool(name="const", bufs=1) as cp,
            tc.tile_pool(name="work", bufs=2) as wp,
            tc.tile_pool(name="sm", bufs=3) as smp,
            tc.tile_pool(name="mm", bufs=4, space="PSUM") as mmp,
            tc.tile_pool(name="agg", bufs=2, space="PSUM") as aggp,
            tc.tile_pool(name="dram", bufs=1, space="DRAM") as drp,
        ):
            h1_loc = drp.tile([cfg.nloc, D1], F32, tag="h1_loc")
            h2_loc = drp.tile([cfg.nloc, D2], F32, tag="h2_loc")

            w1sb = cp.tile([P, K0, D1], F32)
            for k0 in range(K0):
                nc.sync.dma_start(out=w1sb[:, k0, :],
                                  in_=W1[k0 * P:(k0 + 1) * P, :])
            w2sb = cp.tile([P, D2], F32)
            nc.sync.dma_start(out=w2sb[:], in_=W2[:])
            b1sb = cp.tile([P, D1], F32)
            nc.sync.dma_start(out=b1sb[:], in_=b1t[:])
            b2sb = cp.tile([P, D2], F32)
            nc.sync.dma_start(out=b2sb[:], in_=b2t[:])
            dvsb = cp.tile([P, NB], F32)
            nc.sync.dma_start(out=dvsb[:], in_=dinv_t[:])
            iota = cp.tile([P, P], F32)
            nc.gpsimd.iota(iota[:], pattern=[[1, P]], base=0,
                           channel_multiplier=0,
                           allow_small_or_imprecise_dtypes=True)
            ident = cp.tile([P, P], F32)
            from concourse.masks import make_identity
            make_identity(nc, ident[:])

            # ---------------- dense layer 1: h1_loc = dinv * (x @ W1)
            def dense1():
                for b in range(NB):
                    nb = cfg.rows(b)
                    xt = wp.tile([P, K0, P], F32, tag="xt")
                    for k0 in range(K0):
                        nc.sync.dma_start(
                            out=xt[:, k0, :nb],
                            in_=xT[k0 * P:(k0 + 1) * P, b * P:b * P + nb])
                    ps = mmp.tile([P, D1], F32, tag="mm")
                    for k0 in range(K0):
                        nc.tensor.matmul(out=ps[:nb, :], lhsT=xt[:, k0, :nb],
                                         rhs=w1sb[:, k0, :],
                                         start=(k0 == 0), stop=(k0 == K0 - 1))
                    h1w = wp.tile([P, D1], F32, tag="h1w", bufs=3)
                    nc.vector.tensor_scalar(out=h1w[:nb, :], in0=ps[:nb, :],
                                            scalar1=dvsb[:nb, b:b + 1],
                                            scalar2=None, op0=mul)
                    nc.sync.dma_start(out=h1_loc[b * P:b * P + nb, :],
                                      in_=h1w[:nb, :])

            # ---------------- layer-1 aggregation + fused layer-2 dense
            def agg_phase(layer, h_full, h_self, D, out_sink):
                for g in range(NG):
                    Tg = grp_tiles[g]
                    if Tg == 0:
                        continue
                    gt0 = grp_col0[g]
                    stage_raw = wp.tile([P, TG * D1], F32, tag="stage")
                    stage = stage_raw[:].rearrange("p (t d) -> p t d", d=D)
                    idxt = wp.tile([P, TG * 8], I16, tag="idx")
                    drt = wp.tile([P, TG], F32, tag="drel")
                    nc.sync.dma_start(out=idxt[:, :Tg * 8],
                                      in_=idx_t[:, gt0 * 8:(gt0 + Tg) * 8])
                    nc.sync.dma_start(out=drt[:, :Tg],
                                      in_=drel_t[:, gt0:gt0 + Tg])
                    for k in range(NK):
                        c0, ntk = gk_info[g][k]
                        if ntk == 0:
                            continue
                        hi = min((k + 1) * cfg.chunk, cfg.n)
                        nc.gpsimd.dma_gather(
                            out_ap=stage[:, c0:c0 + ntk, :],
                            in_ap=h_full[k * cfg.chunk:hi, :],
                            idxs_ap=idxt[:, c0 * 8:(c0 + ntk) * 8],
                            num_idxs=ntk * P, num_idxs_reg=ntk * P,
                            elem_size=D, single_packet=False)
                    agg = aggp.tile([P, G * D], F32, tag="agg")
                    for slot, b in enumerate(blocks_of_g[g]):
                        segs = blk_tile_cols.get(b, [])
                        ntot = sum(t for _, t, _ in segs)
                        nb = cfg.rows(b)
                        left = ntot
                        first = True
                        for (col, t, k) in segs:
                            sm = smp.tile([P, TBK, P], F32, tag="smulti")
                            for tt in range(t):
                                nc.vector.tensor_scalar(
                                    out=sm[:, tt, :], in0=iota[:],
                                    scalar1=drt[:, col + tt:col + tt + 1],
                                    scalar2=None, op0=eq)
                                nc.tensor.matmul(
                                    out=agg[:, slot * D:(slot + 1) * D],
                                    lhsT=sm[:, tt, :],
                                    rhs=stage[:, col + tt, :],
                                    start=first, stop=(left == 1),
                                    skip_group_check=True)
                                first = False
                                left -= 1
                        # flush
                        selfb = wp.tile([P, D], F32, tag=f"selfb{layer}", bufs=3)
                        nc.sync.dma_start(
                            out=selfb[:nb, :],
                            in_=h_self[b * P:b * P + nb, :])
                        t0 = wp.tile([P, D], F32, tag=f"t0_{layer}", bufs=3)
                        if ntot:
                            nc.vector.tensor_tensor(
                                out=t0[:nb, :],
                                in0=agg[:nb, slot * D:(slot + 1) * D],
                                in1=selfb[:nb, :], op=add)
                        else:
                            nc.vector.tensor_copy(out=t0[:nb, :],
                                                  in_=selfb[:nb, :])
                        out_sink(b, nb, t0)

            def l1_sink(b, nb, t0):
                t1 = wp.tile([P, D1], F32, tag="t1", bufs=3)
                nc.vector.tensor_scalar(out=t1[:nb, :], in0=t0[:nb, :],
                                        scalar1=dvsb[:nb, b:b + 1],
                                        scalar2=None, op0=mul)
                t2 = wp.tile([P, D1], F32, tag="t2", bufs=3)
                nc.vector.tensor_tensor(out=t2[:nb, :], in0=t1[:nb, :],
                                        in1=b1sb[:nb, :], op=add)
                # t3 = dinv * leaky_relu(t2) = max(dinv*t2, alpha*dinv*t2)
                t3s = wp.tile([P, D1], F32, tag="t3s", bufs=3)
                nc.scalar.activation(out=t3s[:nb, :], in_=t2[:nb, :],
                                     func=mybir.ActivationFunctionType.Copy,
                                     bias=0.0, scale=dvsb[:nb, b:b + 1])
                t3u = wp.tile([P, D1], F32, tag="t3u", bufs=3)
                nc.scalar.activation(out=t3u[:nb, :], in_=t3s[:nb, :],
                                     func=mybir.ActivationFunctionType.Copy,
                                     bias=0.0, scale=float(cfg.neg_slope))
                t3 = wp.tile([P, D1], F32, tag="t3", bufs=3)
                nc.vector.tensor_tensor(out=t3[:nb, :], in0=t3s[:nb, :],
                                        in1=t3u[:nb, :],
                                        op=mybir.AluOpType.max)
                tp = mmp.tile([P, P], F32, tag="mm")
                nc.tensor.transpose(out=tp[:, :nb], in_=t3[:nb, :],
                                    identity=ident[:nb, :nb])
                t4 = wp.tile([P, P], F32, tag="t4", bufs=3)
                nc.scalar.copy(out=t4[:, :nb], in_=tp[:, :nb])
                v = mmp.tile([P, D2], F32, tag="mm")
                nc.tensor.matmul(out=v[:nb, :], lhsT=t4[:, :nb],
                                 rhs=w2sb[:], start=True, stop=True)
                h2b = wp.tile([P, D2], F32, tag="h2b", bufs=3)
                nc.vector.tensor_copy(out=h2b[:nb, :], in_=v[:nb, :])
                nc.sync.dma_start(out=h2_loc[b * P:b * P + nb, :],
                                  in_=h2b[:nb, :])

            def l2_sink(b, nb, t0):
                t1 = wp.tile([P, D2], F32, tag="u1", bufs=3)
                nc.vector.tensor_scalar(out=t1[:nb, :], in0=t0[:nb, :],
                                        scalar1=dvsb[:nb, b:b + 1],
                                        scalar2=None, op0=mul)
                t2 = wp.tile([P, D2], F32, tag="u2", bufs=3)
                nc.vector.tensor_tensor(out=t2[:nb, :], in0=t1[:nb, :],
                                        in1=b2sb[:nb, :], op=add)
                nc.sync.dma_start(out=out_t[b * P:b * P + nb, :],
                                  in_=t2[:nb, :])

            mode = getattr(cfg, "repeat_mode", "all")
            h1_full = h2_full = None
            for _rep in range(repeat):
                rep_all = mode == "all" or _rep == 0
                if rep_all or mode == "collectives":
                    h1_full = drp.tile([cfg.n, D1], F32,
                                       tag=f"h1_full{_rep}",
                                       addr_space="Shared",
                                       name=f"h1_full{_rep}")
                    h2_full = drp.tile([cfg.n, D2], F32,
                                       tag=f"h2_full{_rep}",
                                       addr_space="Shared",
                                       name=f"h2_full{_rep}")
                if rep_all:
                    dense1()
                if rep_all or mode == "collectives":
                    nc.gpsimd.collective_compute(
                        "AllGather", mybir.AluOpType.bypass,
                        replica_groups=[list(range(cfg.ncores))],
                        ins=[h1_loc.opt()], outs=[h1_full.opt()])
                if rep_all or mode == "agg":
                    agg_phase(1, h1_full, h1_loc, D1, l1_sink)
                if rep_all or mode == "collectives":
                    nc.gpsimd.collective_compute(
                        "AllGather", mybir.AluOpType.bypass,
                        replica_groups=[list(range(cfg.ncores))],
                        ins=[h2_loc.opt()], outs=[h2_full.opt()])
                if rep_all or mode == "agg":
                    agg_phase(2, h2_full, h2_loc, D2, l2_sink)

    nc.compile()
    return nc


# --------------------------------------------------------------------------
# entry point
# --------------------------------------------------------------------------

_CACHE = {}


def _run(cfg, inputs):
    in_maps, struct = prepare(cfg, inputs["x"], inputs["edge_index"],
                              inputs["W1"], inputs["b1"],
                              inputs["W2"], inputs["b2"])
    key = (cfg.n, cfg.e, struct["T"].tobytes())
    nc = _CACHE.get(key)
    if nc is None:
        nc = build_program(cfg, struct)
        _CACHE[key] = nc
    res = run_bass_kernel_spmd(nc, in_maps, list(range(cfg.ncores)))
    out = np.concatenate([res.results[c]["out_loc"]
                          for c in range(cfg.ncores)], axis=0)
    return out.astype(np.float32)


def kernel(x, edge_index, W1, b1, W2, b2):
    return _run(FULL, dict(x=x, edge_index=edge_index, W1=W1, b1=b1,
                           W2=W2, b2=b2))



# revision 3
# speedup vs baseline: 1.2929x; 1.2929x over previous
"""Two-layer GCN (GCNConv 256->128->64, leaky_relu) on 8 Trainium2 NeuronCores.

Strategy (graph/data parallel per the sharding hint):
  - Nodes are sharded contiguously across the 8 cores (12500 each); each core
    owns the destination rows of the scatter for its node range.
  - Symmetric normalization is folded into node features: with
    h~ = dinv ! (X @ W), the aggregation for node v is
    out[v] = dinv[v] * (sum_{e: dst=v} h~[src_e] + h~[v]) + b.
  - Each layer: dense transform on local nodes -> AllGather h~ (the halo
    exchange; with 8 cores and uniform random edges every core needs nearly
    every node) -> per 128-node dst block, dma_gather the source rows and
    segment-sum them with 0/1 selection-matrix matmuls accumulated in PSUM.
  - Edge tables (gather indices, dst slots) are precomputed host-side from
    edge_index; edges are sorted by (dst block group, src 32k-chunk, dst
    block) and padded to 128-edge tiles.  dma_gather indices are int16, which
    limits each gather to a 32768-row window of the source table - hence the
    4 source chunks.
"""
import math
import sys

import numpy as np

sys.path.insert(0, "/opt/trn_rl_repo")

import concourse.bacc as bacc  # noqa: E402
import concourse.mybir as mybir  # noqa: E402
import concourse.tile as tile  # noqa: E402
from concourse.bass_utils import run_bass_kernel_spmd  # noqa: E402

P = 128
F32 = mybir.dt.float32
I16 = mybir.dt.int16


class Cfg:
    def __init__(self, n, e, d0, d1, d2, ncores=8, grp=6, chunk=32768,
                 neg_slope=0.01):
        assert n % ncores == 0
        self.n, self.e = n, e
        self.d0, self.d1, self.d2 = d0, d1, d2
        self.ncores = ncores
        self.nloc = n // ncores
        self.nblk = math.ceil(self.nloc / P)
        self.grp = grp
        self.ngrp = math.ceil(self.nblk / grp)
        self.chunk = chunk
        self.nchunk = math.ceil(n / chunk)
        self.neg_slope = neg_slope

    def rows(self, b):
        return min(P, self.nloc - b * P)


FULL = Cfg(n=100000, e=1600000, d0=256, d1=128, d2=64)


# --------------------------------------------------------------------------
# host-side preprocessing
# --------------------------------------------------------------------------

def prepare(cfg, x, edge_index, W1, b1, W2, b2):
    """Shard + build per-core tables and the shared tile structure."""
    src = np.asarray(edge_index[0], dtype=np.int64)
    dst = np.asarray(edge_index[1], dtype=np.int64)

    deg = (np.bincount(dst, minlength=cfg.n) + 1).astype(np.float32)
    dinv = (np.float32(1.0) / np.sqrt(deg)).astype(np.float32)

    core = dst // cfg.nloc
    dstl = dst - core * cfg.nloc
    b = dstl >> 7
    k = src // cfg.chunk
    g = b // cfg.grp
    # stream order: core -> g -> k -> b (tiles within segment arbitrary)
    key = ((core * cfg.ngrp + g) * cfg.nchunk + k) * cfg.nblk + b
    order = np.argsort(key, kind="stable")
    src_s, b_s, k_s, core_s, dstl_s = (
        src[order], b[order], k[order], core[order], dstl[order])

    seg_id = ((core_s * cfg.nblk + b_s) * cfg.nchunk + k_s)
    counts = np.bincount(seg_id, minlength=cfg.ncores * cfg.nblk * cfg.nchunk)
    counts = counts.reshape(cfg.ncores, cfg.nblk, cfg.nchunk)
    # shared tile structure: max over cores
    T = np.ceil(counts.max(axis=0) / P).astype(np.int64)  # [nblk, nchunk]

    # segment stream offsets (in padded edge slots), shared across cores
    seg_off = np.zeros((cfg.nblk, cfg.nchunk), np.int64)
    blocks_of_g = [list(range(gg * cfg.grp, min((gg + 1) * cfg.grp, cfg.nblk)))
                   for gg in range(cfg.ngrp)]
    tot_tiles = 0
    for gg in range(cfg.ngrp):
        for kk in range(cfg.nchunk):
            for bb in blocks_of_g[gg]:
                seg_off[bb, kk] = tot_tiles * P
                tot_tiles += int(T[bb, kk])
    etot = tot_tiles * P

    # per-core padded arrays
    xs = np.asarray(x, dtype=np.float32)
    W1 = np.asarray(W1, np.float32)
    W2 = np.asarray(W2, np.float32)
    b1 = np.asarray(b1, np.float32)
    b2 = np.asarray(b2, np.float32)
    b1bc = np.tile(b1[None, :], (P, 1)).astype(np.float32)
    b2bc = np.tile(b2[None, :], (P, 1)).astype(np.float32)

    in_maps = []
    for c in range(cfg.ncores):
        m = core_s == c
        src_c, b_c, k_c, dstl_c = src_s[m], b_s[m], k_s[m], dstl_s[m]
        seg_c = (b_c * cfg.nchunk + k_c)
        # rank within segment: edges are segment-contiguous in stream order
        ne = len(src_c)
        if ne:
            newseg = np.r_[True, seg_c[1:] != seg_c[:-1]]
            seg_start = np.maximum.accumulate(
                np.where(newseg, np.arange(ne), 0))
            pos_in_seg = np.arange(ne) - seg_start
        else:
            pos_in_seg = np.zeros(0, np.int64)
        padded_pos = seg_off[b_c, k_c] + pos_in_seg

        idx_pad = np.zeros(etot, np.int16)
        idx_pad[padded_pos] = (src_c - k_c * cfg.chunk).astype(np.int16)
        dr_pad = np.full(etot, -1.0, np.float32)
        dr_pad[padded_pos] = (dstl_c - b_c * P).astype(np.float32)

        # idx table: wrapped [16, etot//16] (entry [i%16, i//16]), x8 down 128
        idx16 = idx_pad.reshape(etot // 16, 16).T.copy()
        idx_tab = np.tile(idx16, (8, 1))
        # dstrel table: [128, tot_tiles], [p, t] = edge t*128+p
        dstrel_tab = dr_pad.reshape(tot_tiles, P).T.copy()

        lo = c * cfg.nloc
        xT = np.ascontiguousarray(xs[lo:lo + cfg.nloc].T)
        dloc = dinv[lo:lo + cfg.nloc]
        dcols = np.ones(cfg.nblk * P, np.float32)
        dcols[:cfg.nloc] = dloc
        dinv_cols = dcols.reshape(cfg.nblk, P).T.copy()

        in_maps.append({
            "xT": xT, "dinv_cols": dinv_cols,
            "W1": W1, "W2": W2, "b1bc": b1bc, "b2bc": b2bc,
            "idx_tab": idx_tab, "dstrel_tab": dstrel_tab,
        })

    struct = dict(T=T, blocks_of_g=blocks_of_g, tot_tiles=tot_tiles)
    return in_maps, struct


# --------------------------------------------------------------------------
# device program
# --------------------------------------------------------------------------

def build_program(cfg, struct, repeat=1):
    T = struct["T"]
    blocks_of_g = struct["blocks_of_g"]
    tot_tiles = struct["tot_tiles"]
    NB, NK, NG = cfg.nblk, cfg.nchunk, cfg.ngrp
    D0, D1, D2 = cfg.d0, cfg.d1, cfg.d2
    K0 = D0 // P
    G = cfg.grp

    # per-group layout in the tile stream
    grp_tiles = []           # tiles per group
    gk_info = []             # [g][k] -> (col0 within group, ntiles)
    blk_tile_cols = {}       # b -> [(col within group, ntiles, k)]
    grp_col0 = []            # group offset in global tile stream
    tot = 0
    for g in range(NG):
        grp_col0.append(tot)
        col = 0
        ks = []
        for k in range(NK):
            c0 = col
            for b in blocks_of_g[g]:
                t = int(T[b, k])
                if t:
                    blk_tile_cols.setdefault(b, []).append((col, t, k))
                    col += t
            ks.append((c0, col - c0))
        gk_info.append(ks)
        grp_tiles.append(col)
        tot += col
    assert tot == tot_tiles
    TG = max(grp_tiles)
    TBK = max(int(T.max()), 1)

    nq = getattr(cfg, "nqueues", 4)
    nc = bacc.Bacc("TRN2", target_bir_lowering=False, debug=False,
                   num_devices=cfg.ncores, num_swdge_queues=nq)
    qctr = [0]

    def next_q():
        q = qctr[0] % nq
        qctr[0] += 1
        return q
    xT = nc.dram_tensor("xT", [D0, cfg.nloc], F32, kind="ExternalInput")
    dinv_t = nc.dram_tensor("dinv_cols", [P, NB], F32, kind="ExternalInput")
    W1 = nc.dram_tensor("W1", [D0, D1], F32, kind="ExternalInput")
    W2 = nc.dram_tensor("W2", [D1, D2], F32, kind="ExternalInput")
    b1t = nc.dram_tensor("b1bc", [P, D1], F32, kind="ExternalInput")
    b2t = nc.dram_tensor("b2bc", [P, D2], F32, kind="ExternalInput")
    idx_t = nc.dram_tensor("idx_tab", [P, tot_tiles * 8], I16,
                           kind="ExternalInput")
    drel_t = nc.dram_tensor("dstrel_tab", [P, tot_tiles], F32,
                            kind="ExternalInput")
    out_t = nc.dram_tensor("out_loc", [cfg.nloc, D2], F32,
                           kind="ExternalOutput")

    eq = mybir.AluOpType.is_equal
    mul = mybir.AluOpType.mult
    add = mybir.AluOpType.add
    lrelu = mybir.ActivationFunctionType.Lrelu

    with tile.TileContext(nc) as tc:
        with (
            tc.tile_pool(name="const", bufs=1) as cp,
            tc.tile_pool(name="work", bufs=2) as wp,
            tc.tile_pool(name="sm", bufs=3) as smp,
            tc.tile_pool(name="mm", bufs=4, space="PSUM") as mmp,
            tc.tile_pool(name="agg", bufs=2, space="PSUM") as aggp,
            tc.tile_pool(name="dram", bufs=1, space="DRAM") as drp,
        ):
            h1_loc = drp.tile([cfg.nloc, D1], F32, tag="h1_loc")
            h2_loc = drp.tile([cfg.nloc, D2], F32, tag="h2_loc")

            w1sb = cp.tile([P, K0, D1], F32)
            for k0 in range(K0):
                nc.sync.dma_start(out=w1sb[:, k0, :],
                                  in_=W1[k0 * P:(k0 + 1) * P, :])
            w2sb = cp.tile([P, D2], F32)
            nc.sync.dma_start(out=w2sb[:], in_=W2[:])
            b1sb = cp.tile([P, D1], F32)
            nc.sync.dma_start(out=b1sb[:], in_=b1t[:])
            b2sb = cp.tile([P, D2], F32)
            nc.sync.dma_start(out=b2sb[:], in_=b2t[:])
            dvsb = cp.tile([P, NB], F32)
            nc.sync.dma_start(out=dvsb[:], in_=dinv_t[:])
            iota = cp.tile([P, P], F32)
            nc.gpsimd.iota(iota[:], pattern=[[1, P]], base=0,
                           channel_multiplier=0,
                           allow_small_or_imprecise_dtypes=True)
            ident = cp.tile([P, P], F32)
            from concourse.masks import make_identity
            make_identity(nc, ident[:])

            # ---------------- dense layer 1: h1_loc = dinv * (x @ W1)
            def dense1():
                for b in range(NB):
                    nb = cfg.rows(b)
                    xt = wp.tile([P, K0, P], F32, tag="xt")
                    for k0 in range(K0):
                        nc.sync.dma_start(
                            out=xt[:, k0, :nb],
                            in_=xT[k0 * P:(k0 + 1) * P, b * P:b * P + nb])
                    ps = mmp.tile([P, D1], F32, tag="mm")
                    for k0 in range(K0):
                        nc.tensor.matmul(out=ps[:nb, :], lhsT=xt[:, k0, :nb],
                                         rhs=w1sb[:, k0, :],
                                         start=(k0 == 0), stop=(k0 == K0 - 1))
                    h1w = wp.tile([P, D1], F32, tag="h1w", bufs=3)
                    nc.vector.tensor_scalar(out=h1w[:nb, :], in0=ps[:nb, :],
                                            scalar1=dvsb[:nb, b:b + 1],
                                            scalar2=None, op0=mul)
                    nc.sync.dma_start(out=h1_loc[b * P:b * P + nb, :],
                                      in_=h1w[:nb, :])

            # ---------------- layer-1 aggregation + fused layer-2 dense
            def agg_phase(layer, h_full, h_self, D, out_sink):
                for g in range(NG):
                    Tg = grp_tiles[g]
                    if Tg == 0:
                        continue
                    gt0 = grp_col0[g]
                    stage_raw = wp.tile([P, TG * D1], F32, tag="stage")
                    stage = stage_raw[:].rearrange("p (t d) -> p t d", d=D)
                    idxt = wp.tile([P, TG * 8], I16, tag="idx")
                    drt = wp.tile([P, TG], F32, tag="drel")
                    nc.sync.dma_start(out=idxt[:, :Tg * 8],
                                      in_=idx_t[:, gt0 * 8:(gt0 + Tg) * 8])
                    nc.sync.dma_start(out=drt[:, :Tg],
                                      in_=drel_t[:, gt0:gt0 + Tg])
                    for k in range(NK):
                        c0, ntk = gk_info[g][k]
                        if ntk == 0:
                            continue
                        hi = min((k + 1) * cfg.chunk, cfg.n)
                        nc.gpsimd.dma_gather(
                            out_ap=stage[:, c0:c0 + ntk, :],
                            in_ap=h_full[k * cfg.chunk:hi, :],
                            idxs_ap=idxt[:, c0 * 8:(c0 + ntk) * 8],
                            num_idxs=ntk * P, num_idxs_reg=ntk * P,
                            elem_size=D, single_packet=False,
                            queue_num=next_q())
                    agg = aggp.tile([P, G * D], F32, tag="agg")
                    for slot, b in enumerate(blocks_of_g[g]):
                        segs = blk_tile_cols.get(b, [])
                        ntot = sum(t for _, t, _ in segs)
                        nb = cfg.rows(b)
                        left = ntot
                        first = True
                        for (col, t, k) in segs:
                            sm = smp.tile([P, TBK, P], F32, tag="smulti")
                            for tt in range(t):
                                nc.vector.tensor_scalar(
                                    out=sm[:, tt, :], in0=iota[:],
                                    scalar1=drt[:, col + tt:col + tt + 1],
                                    scalar2=None, op0=eq)
                                nc.tensor.matmul(
                                    out=agg[:, slot * D:(slot + 1) * D],
                                    lhsT=sm[:, tt, :],
                                    rhs=stage[:, col + tt, :],
                                    start=first, stop=(left == 1),
                                    skip_group_check=True)
                                first = False
                                left -= 1
                        # flush
                        selfb = wp.tile([P, D], F32, tag=f"selfb{layer}", bufs=3)
                        nc.sync.dma_start(
                            out=selfb[:nb, :],
                            in_=h_self[b * P:b * P + nb, :])
                        t0 = wp.tile([P, D], F32, tag=f"t0_{layer}", bufs=3)
                        if ntot:
                            nc.vector.tensor_tensor(
                                out=t0[:nb, :],
                                in0=agg[:nb, slot * D:(slot + 1) * D],
                                in1=selfb[:nb, :], op=add)
                        else:
                            nc.vector.tensor_copy(out=t0[:nb, :],
                                                  in_=selfb[:nb, :])
                        out_sink(b, nb, t0)

            def l1_sink(b, nb, t0):
                t1 = wp.tile([P, D1], F32, tag="t1", bufs=3)
                nc.vector.tensor_scalar(out=t1[:nb, :], in0=t0[:nb, :],
                                        scalar1=dvsb[:nb, b:b + 1],
                                        scalar2=None, op0=mul)
                t2 = wp.tile([P, D1], F32, tag="t2", bufs=3)
                nc.vector.tensor_tensor(out=t2[:nb, :], in0=t1[:nb, :],
                                        in1=b1sb[:nb, :], op=add)
                # t3 = dinv * leaky_relu(t2) = max(dinv*t2, alpha*dinv*t2)
                t3s = wp.tile([P, D1], F32, tag="t3s", bufs=3)
                nc.scalar.activation(out=t3s[:nb, :], in_=t2[:nb, :],
                                     func=mybir.ActivationFunctionType.Copy,
                                     bias=0.0, scale=dvsb[:nb, b:b + 1])
                t3u = wp.tile([P, D1], F32, tag="t3u", bufs=3)
                nc.scalar.activation(out=t3u[:nb, :], in_=t3s[:nb, :],
                                     func=mybir.ActivationFunctionType.Copy,
                                     bias=0.0, scale=float(cfg.neg_slope))
                t3 = wp.tile([P, D1], F32, tag="t3", bufs=3)
                nc.vector.tensor_tensor(out=t3[:nb, :], in0=t3s[:nb, :],
                                        in1=t3u[:nb, :],
                                        op=mybir.AluOpType.max)
                tp = mmp.tile([P, P], F32, tag="mm")
                nc.tensor.transpose(out=tp[:, :nb], in_=t3[:nb, :],
                                    identity=ident[:nb, :nb])
                t4 = wp.tile([P, P], F32, tag="t4", bufs=3)
                nc.scalar.copy(out=t4[:, :nb], in_=tp[:, :nb])
                v = mmp.tile([P, D2], F32, tag="mm")
                nc.tensor.matmul(out=v[:nb, :], lhsT=t4[:, :nb],
                                 rhs=w2sb[:], start=True, stop=True)
                h2b = wp.tile([P, D2], F32, tag="h2b", bufs=3)
                nc.vector.tensor_copy(out=h2b[:nb, :], in_=v[:nb, :])
                nc.sync.dma_start(out=h2_loc[b * P:b * P + nb, :],
                                  in_=h2b[:nb, :])

            def l2_sink(b, nb, t0):
                t1 = wp.tile([P, D2], F32, tag="u1", bufs=3)
                nc.vector.tensor_scalar(out=t1[:nb, :], in0=t0[:nb, :],
                                        scalar1=dvsb[:nb, b:b + 1],
                                        scalar2=None, op0=mul)
                t2 = wp.tile([P, D2], F32, tag="u2", bufs=3)
                nc.vector.tensor_tensor(out=t2[:nb, :], in0=t1[:nb, :],
                                        in1=b2sb[:nb, :], op=add)
                nc.sync.dma_start(out=out_t[b * P:b * P + nb, :],
                                  in_=t2[:nb, :])

            mode = getattr(cfg, "repeat_mode", "all")
            h1_full = h2_full = None
            for _rep in range(repeat):
                rep_all = mode == "all" or _rep == 0
                if rep_all or mode == "collectives":
                    h1_full = drp.tile([cfg.n, D1], F32,
                                       tag=f"h1_full{_rep}",
                                       addr_space="Shared",
                                       name=f"h1_full{_rep}")
                    h2_full = drp.tile([cfg.n, D2], F32,
                                       tag=f"h2_full{_rep}",
                                       addr_space="Shared",
                                       name=f"h2_full{_rep}")
                if rep_all:
                    dense1()
                if rep_all or mode == "collectives":
                    nc.gpsimd.collective_compute(
                        "AllGather", mybir.AluOpType.bypass,
                        replica_groups=[list(range(cfg.ncores))],
                        ins=[h1_loc.opt()], outs=[h1_full.opt()])
                if rep_all or mode == "agg":
                    agg_phase(1, h1_full, h1_loc, D1, l1_sink)
                if rep_all or mode == "collectives":
                    nc.gpsimd.collective_compute(
                        "AllGather", mybir.AluOpType.bypass,
                        replica_groups=[list(range(cfg.ncores))],
                        ins=[h2_loc.opt()], outs=[h2_full.opt()])
                if rep_all or mode == "agg":
                    agg_phase(2, h2_full, h2_loc, D2, l2_sink)

    nc.compile()
    return nc


# --------------------------------------------------------------------------
# entry point
# --------------------------------------------------------------------------

_CACHE = {}


def _run(cfg, inputs):
    in_maps, struct = prepare(cfg, inputs["x"], inputs["edge_index"],
                              inputs["W1"], inputs["b1"],
                              inputs["W2"], inputs["b2"])
    key = (cfg.n, cfg.e, struct["T"].tobytes())
    nc = _CACHE.get(key)
    if nc is None:
        nc = build_program(cfg, struct)
        _CACHE[key] = nc
    res = run_bass_kernel_spmd(nc, in_maps, list(range(cfg.ncores)))
    out = np.concatenate([res.results[c]["out_loc"]
                          for c in range(cfg.ncores)], axis=0)
    return out.astype(np.float32)


def kernel(x, edge_index, W1, b1, W2, b2):
    return _run(FULL, dict(x=x, edge_index=edge_index, W1=W1, b1=b1,
                           W2=W2, b2=b2))



# revision 15
# speedup vs baseline: 1.9559x; 1.5127x over previous
"""Two-layer GCN (GCNConv 256->128->64, leaky_relu) on 8 Trainium2 NeuronCores.

Strategy (graph/data parallel per the sharding hint):
  - Nodes are sharded contiguously across the 8 cores (12500 each); each core
    owns the destination rows of the scatter for its node range.
  - Symmetric normalization is folded into node features: with
    h~ = dinv ! (X @ W), the aggregation for node v is
    out[v] = dinv[v] * (sum_{e: dst=v} h~[src_e] + h~[v]) + b.
  - Each layer: dense transform on local nodes -> AllGather h~ (the halo
    exchange; with 8 cores and uniform random edges every core needs nearly
    every node) -> per 128-node dst block, dma_gather the source rows and
    segment-sum them with 0/1 selection-matrix matmuls accumulated in PSUM.
  - Edge tables (gather indices, dst slots) are precomputed host-side from
    edge_index; edges are sorted by (dst block group, src 32k-chunk, dst
    block) and padded to 128-edge tiles.  dma_gather indices are int16, which
    limits each gather to a 32768-row window of the source table - hence the
    4 source chunks.
"""
import math
import sys

import numpy as np

sys.path.insert(0, "/opt/trn_rl_repo")

import concourse.bacc as bacc  # noqa: E402
import concourse.mybir as mybir  # noqa: E402
import concourse.tile as tile  # noqa: E402
from concourse.bass_utils import run_bass_kernel_spmd  # noqa: E402

P = 128
F32 = mybir.dt.float32
I16 = mybir.dt.int16


class Cfg:
    def __init__(self, n, e, d0, d1, d2, ncores=8, grp=6, chunk=32768,
                 neg_slope=0.01):
        assert n % ncores == 0
        self.n, self.e = n, e
        self.d0, self.d1, self.d2 = d0, d1, d2
        self.ncores = ncores
        self.nloc = n // ncores
        self.nblk = math.ceil(self.nloc / P)
        self.grp = grp
        self.ngrp = math.ceil(self.nblk / grp)
        self.chunk = chunk
        self.nchunk = math.ceil(n / chunk)
        self.neg_slope = neg_slope

    def rows(self, b):
        return min(P, self.nloc - b * P)


FULL = Cfg(n=100000, e=1600000, d0=256, d1=128, d2=64)


# --------------------------------------------------------------------------
# host-side preprocessing
# --------------------------------------------------------------------------

def prepare(cfg, x, edge_index, W1, b1, W2, b2):
    """Shard + build per-core tables and the shared tile structure."""
    src = np.asarray(edge_index[0], dtype=np.int64)
    dst = np.asarray(edge_index[1], dtype=np.int64)

    deg = (np.bincount(dst, minlength=cfg.n) + 1).astype(np.float32)
    dinv = (np.float32(1.0) / np.sqrt(deg)).astype(np.float32)

    core = dst // cfg.nloc
    dstl = dst - core * cfg.nloc
    b = dstl >> 7
    k = src // cfg.chunk
    g = b // cfg.grp
    # stream order: core -> g -> k -> b (tiles within segment arbitrary)
    key = ((core * cfg.ngrp + g) * cfg.nchunk + k) * cfg.nblk + b
    order = np.argsort(key, kind="stable")
    src_s, b_s, k_s, core_s, dstl_s = (
        src[order], b[order], k[order], core[order], dstl[order])

    seg_id = ((core_s * cfg.nblk + b_s) * cfg.nchunk + k_s)
    counts = np.bincount(seg_id, minlength=cfg.ncores * cfg.nblk * cfg.nchunk)
    counts = counts.reshape(cfg.ncores, cfg.nblk, cfg.nchunk)
    # shared tile structure: max over cores
    T = np.ceil(counts.max(axis=0) / P).astype(np.int64)  # [nblk, nchunk]

    # segment stream offsets (in padded edge slots), shared across cores
    seg_off = np.zeros((cfg.nblk, cfg.nchunk), np.int64)
    blocks_of_g = [list(range(gg * cfg.grp, min((gg + 1) * cfg.grp, cfg.nblk)))
                   for gg in range(cfg.ngrp)]
    tot_tiles = 0
    for gg in range(cfg.ngrp):
        for kk in range(cfg.nchunk):
            for bb in blocks_of_g[gg]:
                seg_off[bb, kk] = tot_tiles * P
                tot_tiles += int(T[bb, kk])
    etot = tot_tiles * P

    # per-core padded arrays
    xs = np.asarray(x, dtype=np.float32)
    W1 = np.asarray(W1, np.float32)
    W2 = np.asarray(W2, np.float32)
    b1 = np.asarray(b1, np.float32)
    b2 = np.asarray(b2, np.float32)
    b1bc = np.tile(b1[None, :], (P, 1)).astype(np.float32)
    b2bc = np.tile(b2[None, :], (P, 1)).astype(np.float32)

    in_maps = []
    for c in range(cfg.ncores):
        m = core_s == c
        src_c, b_c, k_c, dstl_c = src_s[m], b_s[m], k_s[m], dstl_s[m]
        seg_c = (b_c * cfg.nchunk + k_c)
        # rank within segment: edges are segment-contiguous in stream order
        ne = len(src_c)
        if ne:
            newseg = np.r_[True, seg_c[1:] != seg_c[:-1]]
            seg_start = np.maximum.accumulate(
                np.where(newseg, np.arange(ne), 0))
            pos_in_seg = np.arange(ne) - seg_start
        else:
            pos_in_seg = np.zeros(0, np.int64)
        padded_pos = seg_off[b_c, k_c] + pos_in_seg

        idx_pad = np.zeros(etot, np.int16)
        idx_pad[padded_pos] = (src_c - k_c * cfg.chunk).astype(np.int16)
        dr_pad = np.full(etot, -1.0, np.float32)
        dr_pad[padded_pos] = (dstl_c - b_c * P).astype(np.float32)

        # idx table: wrapped [16, etot//16] (entry [i%16, i//16]), x8 down 128
        idx16 = idx_pad.reshape(etot // 16, 16).T.copy()
        idx_tab = np.tile(idx16, (8, 1))
        # dstrel table: [128, tot_tiles], [p, t] = edge t*128+p
        dstrel_tab = dr_pad.reshape(tot_tiles, P).T.copy()

        lo = c * cfg.nloc
        xT = np.ascontiguousarray(xs[lo:lo + cfg.nloc].T)
        dloc = dinv[lo:lo + cfg.nloc]
        dcols = np.ones(cfg.nblk * P, np.float32)
        dcols[:cfg.nloc] = dloc
        dinv_cols = dcols.reshape(cfg.nblk, P).T.copy()

        in_maps.append({
            "xT": xT, "dinv_cols": dinv_cols,
            "W1": W1, "W2": W2, "b1bc": b1bc, "b2bc": b2bc,
            "idx_tab": idx_tab, "dstrel_tab": dstrel_tab,
        })

    struct = dict(T=T, blocks_of_g=blocks_of_g, tot_tiles=tot_tiles)
    return in_maps, struct


# --------------------------------------------------------------------------
# device program
# --------------------------------------------------------------------------

def build_program(cfg, struct, repeat=1):
    T = struct["T"]
    blocks_of_g = struct["blocks_of_g"]
    tot_tiles = struct["tot_tiles"]
    NB, NK, NG = cfg.nblk, cfg.nchunk, cfg.ngrp
    D0, D1, D2 = cfg.d0, cfg.d1, cfg.d2
    K0 = D0 // P
    G = cfg.grp

    # per-group layout in the tile stream
    grp_tiles = []           # tiles per group
    gk_info = []             # [g][k] -> (col0 within group, ntiles)
    blk_tile_cols = {}       # b -> [(col within group, ntiles, k)]
    grp_col0 = []            # group offset in global tile stream
    tot = 0
    for g in range(NG):
        grp_col0.append(tot)
        col = 0
        ks = []
        for k in range(NK):
            c0 = col
            for b in blocks_of_g[g]:
                t = int(T[b, k])
                if t:
                    blk_tile_cols.setdefault(b, []).append((col, t, k))
                    col += t
            ks.append((c0, col - c0))
        gk_info.append(ks)
        grp_tiles.append(col)
        tot += col
    assert tot == tot_tiles
    TG = max(grp_tiles)
    TBK = max(int(T.max()), 1)

    nq = getattr(cfg, "nqueues", 4)
    nc = bacc.Bacc("TRN2", target_bir_lowering=False, debug=False,
                   num_devices=cfg.ncores, num_swdge_queues=nq)
    qctr = [0]

    def next_q():
        q = qctr[0] % nq
        qctr[0] += 1
        return q
    xT = nc.dram_tensor("xT", [D0, cfg.nloc], F32, kind="ExternalInput")
    dinv_t = nc.dram_tensor("dinv_cols", [P, NB], F32, kind="ExternalInput")
    W1 = nc.dram_tensor("W1", [D0, D1], F32, kind="ExternalInput")
    W2 = nc.dram_tensor("W2", [D1, D2], F32, kind="ExternalInput")
    b1t = nc.dram_tensor("b1bc", [P, D1], F32, kind="ExternalInput")
    b2t = nc.dram_tensor("b2bc", [P, D2], F32, kind="ExternalInput")
    idx_t = nc.dram_tensor("idx_tab", [P, tot_tiles * 8], I16,
                           kind="ExternalInput")
    drel_t = nc.dram_tensor("dstrel_tab", [P, tot_tiles], F32,
                            kind="ExternalInput")
    out_t = nc.dram_tensor("out_loc", [cfg.nloc, D2], F32,
                           kind="ExternalOutput")

    eq = mybir.AluOpType.is_equal
    mul = mybir.AluOpType.mult
    add = mybir.AluOpType.add
    lrelu = mybir.ActivationFunctionType.Lrelu

    with tile.TileContext(nc) as tc:
        with (
            tc.tile_pool(name="const", bufs=1) as cp,
            tc.tile_pool(name="work", bufs=2) as wp,
            tc.tile_pool(name="sm", bufs=3) as smp,
            tc.tile_pool(name="mm", bufs=4, space="PSUM") as mmp,
            tc.tile_pool(name="agg", bufs=2, space="PSUM") as aggp,
            tc.tile_pool(name="dram", bufs=1, space="DRAM") as drp,
        ):
            h1_loc = drp.tile([cfg.nloc, D1], F32, tag="h1_loc")
            h2_loc = drp.tile([cfg.nloc, D2], F32, tag="h2_loc")

            w1sb = cp.tile([P, K0, D1], F32)
            for k0 in range(K0):
                nc.sync.dma_start(out=w1sb[:, k0, :],
                                  in_=W1[k0 * P:(k0 + 1) * P, :])
            w2sb = cp.tile([P, D2], F32)
            nc.sync.dma_start(out=w2sb[:], in_=W2[:])
            b1sb = cp.tile([P, D1], F32)
            nc.sync.dma_start(out=b1sb[:], in_=b1t[:])
            b2sb = cp.tile([P, D2], F32)
            nc.sync.dma_start(out=b2sb[:], in_=b2t[:])
            dvsb = cp.tile([P, NB], F32)
            nc.sync.dma_start(out=dvsb[:], in_=dinv_t[:])
            iota = cp.tile([P, P], F32)
            nc.gpsimd.iota(iota[:], pattern=[[1, P]], base=0,
                           channel_multiplier=0,
                           allow_small_or_imprecise_dtypes=True)
            ident = cp.tile([P, P], F32)
            from concourse.masks import make_identity
            make_identity(nc, ident[:])

            # ---------------- dense layer 1: h1_loc = dinv * (x @ W1)
            def dense1():
                for b in range(NB):
                    nb = cfg.rows(b)
                    xt = wp.tile([P, K0, P], F32, tag="xt")
                    for k0 in range(K0):
                        nc.sync.dma_start(
                            out=xt[:, k0, :nb],
                            in_=xT[k0 * P:(k0 + 1) * P, b * P:b * P + nb])
                    ps = mmp.tile([P, D1], F32, tag="mm")
                    for k0 in range(K0):
                        nc.tensor.matmul(out=ps[:nb, :], lhsT=xt[:, k0, :nb],
                                         rhs=w1sb[:, k0, :],
                                         start=(k0 == 0), stop=(k0 == K0 - 1))
                    h1w = wp.tile([P, D1], F32, tag="h1w", bufs=3)
                    nc.vector.tensor_scalar(out=h1w[:nb, :], in0=ps[:nb, :],
                                            scalar1=dvsb[:nb, b:b + 1],
                                            scalar2=None, op0=mul)
                    nc.sync.dma_start(out=h1_loc[b * P:b * P + nb, :],
                                      in_=h1w[:nb, :])

            # ---------------- layer-1 aggregation + fused layer-2 dense
            def agg_phase(layer, h_full, h_self, D, out_sink,
                          do_gather=True, do_compute=True, gvar=0):
                for g in range(NG):
                    Tg = grp_tiles[g]
                    if Tg == 0:
                        continue
                    gt0 = grp_col0[g]
                    stage_raw = wp.tile([P, (TG + (TG & 1)) * D1], F32,
                                        tag="stage")
                    stage = stage_raw[:].rearrange("p (t d) -> p t d", d=D)
                    idxt = wp.tile([P, TG * 8], I16, tag="idx")
                    drt = wp.tile([P, TG], F32, tag="drel")
                    nc.sync.dma_start(out=idxt[:, :Tg * 8],
                                      in_=idx_t[:, gt0 * 8:(gt0 + Tg) * 8])
                    nc.sync.dma_start(out=drt[:, :Tg],
                                      in_=drel_t[:, gt0:gt0 + Tg])
                    if not do_gather or gvar:
                        nc.gpsimd.memset(stage_raw[:, :Tg * D], 0.0)
                    if do_gather:
                        for k in range(NK):
                            c0, ntk = gk_info[g][k]
                            if ntk == 0:
                                continue
                            hi = min((k + 1) * cfg.chunk, cfg.n)
                            if gvar in (0, 1, 4):
                                nc.gpsimd.dma_gather(
                                    out_ap=stage[:, c0:c0 + ntk, :],
                                    in_ap=h_full[k * cfg.chunk:hi, :],
                                    idxs_ap=idxt[:, c0 * 8:(c0 + ntk) * 8],
                                    num_idxs=ntk * P, num_idxs_reg=ntk * P,
                                    elem_size=D,
                                    single_packet=(gvar == 4),
                                    queue_num=next_q())
                            elif gvar == 2:
                                # same desc count, half bytes (timing probe)
                                in64 = h_full[:].rearrange(
                                    "n (x d) -> (n x) d", x=2)
                                st64 = stage_raw[:].rearrange(
                                    "p (t d) -> p t d", d=D // 2)
                                nc.gpsimd.dma_gather(
                                    out_ap=st64[:, c0:c0 + ntk, :],
                                    in_ap=in64[:cfg.chunk, :],
                                    idxs_ap=idxt[:, c0 * 8:(c0 + ntk) * 8],
                                    num_idxs=ntk * P, num_idxs_reg=ntk * P,
                                    elem_size=D // 2, single_packet=False,
                                    queue_num=next_q())
                            elif gvar == 3:
                                # half desc count, same bytes (timing probe)
                                n2 = ntk // 2
                                if n2 == 0:
                                    continue
                                in256 = h_full[:].rearrange(
                                    "(r x) d -> r (x d)", x=2)
                                st2 = stage_raw[:].rearrange(
                                    "p (t d) -> p t d", d=2 * D)
                                c02 = c0 // 2
                                nc.gpsimd.dma_gather(
                                    out_ap=st2[:, c02:c02 + n2, :],
                                    in_ap=in256[:cfg.chunk, :],
                                    idxs_ap=idxt[:, c0 * 8:c0 * 8 + n2 * 8],
                                    num_idxs=n2 * P, num_idxs_reg=n2 * P,
                                    elem_size=2 * D, single_packet=False,
                                    queue_num=next_q())
                    if not do_compute:
                        continue
                    agg = aggp.tile([P, G * D], F32, tag="agg")
                    for slot, b in enumerate(blocks_of_g[g]):
                        segs = blk_tile_cols.get(b, [])
                        ntot = sum(t for _, t, _ in segs)
                        nb = cfg.rows(b)
                        left = ntot
                        first = True
                        for (col, t, k) in segs:
                            sm = smp.tile([P, TBK, P], F32, tag="smulti")
                            for tt in range(t):
                                nc.vector.tensor_scalar(
                                    out=sm[:, tt, :], in0=iota[:],
                                    scalar1=drt[:, col + tt:col + tt + 1],
                                    scalar2=None, op0=eq)
                                nc.tensor.matmul(
                                    out=agg[:, slot * D:(slot + 1) * D],
                                    lhsT=sm[:, tt, :],
                                    rhs=stage[:, col + tt, :],
                                    start=first, stop=(left == 1),
                                    skip_group_check=True)
                                first = False
                                left -= 1
                        # flush
                        selfb = wp.tile([P, D], F32, tag=f"selfb{layer}", bufs=3)
                        nc.sync.dma_start(
                            out=selfb[:nb, :],
                            in_=h_self[b * P:b * P + nb, :])
                        t0 = wp.tile([P, D], F32, tag=f"t0_{layer}", bufs=3)
                        if ntot:
                            nc.vector.tensor_tensor(
                                out=t0[:nb, :],
                                in0=agg[:nb, slot * D:(slot + 1) * D],
                                in1=selfb[:nb, :], op=add)
                        else:
                            nc.vector.tensor_copy(out=t0[:nb, :],
                                                  in_=selfb[:nb, :])
                        out_sink(b, nb, t0)

            def l1_sink(b, nb, t0):
                t1 = wp.tile([P, D1], F32, tag="t1", bufs=3)
                nc.vector.tensor_scalar(out=t1[:nb, :], in0=t0[:nb, :],
                                        scalar1=dvsb[:nb, b:b + 1],
                                        scalar2=None, op0=mul)
                t2 = wp.tile([P, D1], F32, tag="t2", bufs=3)
                nc.vector.tensor_tensor(out=t2[:nb, :], in0=t1[:nb, :],
                                        in1=b1sb[:nb, :], op=add)
                # t3 = dinv * leaky_relu(t2) = max(dinv*t2, alpha*dinv*t2)
                t3s = wp.tile([P, D1], F32, tag="t3s", bufs=3)
                nc.scalar.activation(out=t3s[:nb, :], in_=t2[:nb, :],
                                     func=mybir.ActivationFunctionType.Copy,
                                     bias=0.0, scale=dvsb[:nb, b:b + 1])
                t3u = wp.tile([P, D1], F32, tag="t3u", bufs=3)
                nc.scalar.activation(out=t3u[:nb, :], in_=t3s[:nb, :],
                                     func=mybir.ActivationFunctionType.Copy,
                                     bias=0.0, scale=float(cfg.neg_slope))
                t3 = wp.tile([P, D1], F32, tag="t3", bufs=3)
                nc.vector.tensor_tensor(out=t3[:nb, :], in0=t3s[:nb, :],
                                        in1=t3u[:nb, :],
                                        op=mybir.AluOpType.max)
                tp = mmp.tile([P, P], F32, tag="mm")
                nc.tensor.transpose(out=tp[:, :nb], in_=t3[:nb, :],
                                    identity=ident[:nb, :nb])
                t4 = wp.tile([P, P], F32, tag="t4", bufs=3)
                nc.scalar.copy(out=t4[:, :nb], in_=tp[:, :nb])
                v = mmp.tile([P, D2], F32, tag="mm")
                nc.tensor.matmul(out=v[:nb, :], lhsT=t4[:, :nb],
                                 rhs=w2sb[:], start=True, stop=True)
                h2b = wp.tile([P, D2], F32, tag="h2b", bufs=3)
                nc.vector.tensor_copy(out=h2b[:nb, :], in_=v[:nb, :])
                nc.sync.dma_start(out=h2_loc[b * P:b * P + nb, :],
                                  in_=h2b[:nb, :])

            def l2_sink(b, nb, t0):
                t1 = wp.tile([P, D2], F32, tag="u1", bufs=3)
                nc.vector.tensor_scalar(out=t1[:nb, :], in0=t0[:nb, :],
                                        scalar1=dvsb[:nb, b:b + 1],
                                        scalar2=None, op0=mul)
                t2 = wp.tile([P, D2], F32, tag="u2", bufs=3)
                nc.vector.tensor_tensor(out=t2[:nb, :], in0=t1[:nb, :],
                                        in1=b2sb[:nb, :], op=add)
                nc.sync.dma_start(out=out_t[b * P:b * P + nb, :],
                                  in_=t2[:nb, :])

            mode = getattr(cfg, "repeat_mode", "all")
            h1_full = h2_full = None
            for _rep in range(repeat):
                rep_all = mode == "all" or _rep == 0
                if rep_all or mode == "collectives":
                    h1_full = drp.tile([cfg.n, D1], F32,
                                       tag=f"h1_full{_rep}",
                                       addr_space="Shared",
                                       name=f"h1_full{_rep}")
                    h2_full = drp.tile([cfg.n, D2], F32,
                                       tag=f"h2_full{_rep}",
                                       addr_space="Shared",
                                       name=f"h2_full{_rep}")
                if rep_all:
                    dense1()
                if rep_all or mode == "collectives":
                    nc.gpsimd.collective_compute(
                        "AllGather", mybir.AluOpType.bypass,
                        replica_groups=[list(range(cfg.ncores))],
                        ins=[h1_loc.opt()], outs=[h1_full.opt()])
                if rep_all or mode in ("agg", "a1"):
                    agg_phase(1, h1_full, h1_loc, D1, l1_sink)
                elif mode == "g1":
                    agg_phase(1, h1_full, h1_loc, D1, l1_sink,
                              do_compute=False)
                elif mode == "c1":
                    agg_phase(1, h1_full, h1_loc, D1, l1_sink,
                              do_gather=False)
                elif mode in ("g0", "g2", "g3"):
                    agg_phase(1, h1_full, h1_loc, D1, l1_sink,
                              gvar={"g0": 1, "g2": 2, "g3": 3}[mode])
                elif mode == "sp":
                    agg_phase(1, h1_full, h1_loc, D1, l1_sink, gvar=4)
                elif mode == "d1":
                    dense1()
                if rep_all or mode == "collectives":
                    nc.gpsimd.collective_compute(
                        "AllGather", mybir.AluOpType.bypass,
                        replica_groups=[list(range(cfg.ncores))],
                        ins=[h2_loc.opt()], outs=[h2_full.opt()])
                if rep_all or mode == "agg":
                    agg_phase(2, h2_full, h2_loc, D2, l2_sink)

    nc.compile()
    return nc


# --------------------------------------------------------------------------
# entry point
# --------------------------------------------------------------------------

_CACHE = {}


def _run(cfg, inputs):
    in_maps, struct = prepare(cfg, inputs["x"], inputs["edge_index"],
                              inputs["W1"], inputs["b1"],
                              inputs["W2"], inputs["b2"])
    key = (cfg.n, cfg.e, struct["T"].tobytes())
    nc = _CACHE.get(key)
    if nc is None:
        nc = build_program(cfg, struct)
        _CACHE[key] = nc
    res = run_bass_kernel_spmd(nc, in_maps, list(range(cfg.ncores)))
    out = np.concatenate([res.results[c]["out_loc"]
                          for c in range(cfg.ncores)], axis=0)
    return out.astype(np.float32)


def kernel(x, edge_index, W1, b1, W2, b2):
    return _run(FULL, dict(x=x, edge_index=edge_index, W1=W1, b1=b1,
                           W2=W2, b2=b2))



# revision 17
# speedup vs baseline: 3.9374x; 2.0131x over previous
"""Two-layer GCN (GCNConv 256->128->64, leaky_relu) on 8 Trainium2 cores, v2.

Structure (per core, nodes sharded contiguously, 12500 each):
  dense1:  h1 = dinv * (x @ W1)                  [bf16, 128 wide]
  AG1:     AllGather h1 in 4 node-range chunks (3125 local rows each) ->
           h1c_k [25000, 128] bf16, rows core-major per chunk; emitted as
           soon as the dense blocks covering the chunk complete.
  agg1:    per group of 6 dst blocks: dma_gather per-edge source rows from
           the 4 chunk tensors; segment-sum via selection-matrix matmuls
           (eq against a group-relative iota) accumulated in PSUM; sink
           fuses +self, dinv, +b1, leaky-relu and next-layer dinv -> h2 bf16.
  AG2:     chunked AllGather of h2, emitted as agg1 groups complete.
  agg2:    same aggregation of h2; sink applies dinv then W2 and b2
           ((A h) W2 = A (h W2) by linearity) -> out f32.

Edge stream: sorted (core, group, chunk, block), packed contiguously per
(group, chunk) with the tile count = max over cores; each core pads its own
tail with idx=-1 (dma_gather skips trailing negative indices -> descriptor
count tracks the core's real edge count). Tiles may straddle dst blocks;
each tile matmuls into every slot any core's copy can touch, with the
per-core dstrel data making the selection exact.
"""
import math
import sys

import numpy as np

sys.path.insert(0, "/opt/trn_rl_repo")

import concourse.bacc as bacc  # noqa: E402
import concourse.mybir as mybir  # noqa: E402
import concourse.tile as tile  # noqa: E402
from concourse.bass_utils import run_bass_kernel_spmd  # noqa: E402

P = 128
F32 = mybir.dt.float32
BF16 = mybir.dt.bfloat16
I16 = mybir.dt.int16

try:
    import ml_dtypes
    NP_BF16 = ml_dtypes.bfloat16
except ImportError:  # pragma: no cover
    import jax.numpy as jnp
    NP_BF16 = jnp.bfloat16


class Cfg:
    def __init__(self, n, e, d0, d1, d2, ncores=8, grp=6, neg_slope=0.01):
        assert n % ncores == 0
        self.n, self.e = n, e
        self.d0, self.d1, self.d2 = d0, d1, d2
        self.ncores = ncores
        self.nloc = n // ncores
        self.nblk = math.ceil(self.nloc / P)
        self.grp = grp
        self.ngrp = math.ceil(self.nblk / grp)
        self.nchunk = 4
        self.lchunk = self.nloc // self.nchunk     # local rows per AG chunk
        self.gchunk = self.lchunk * ncores         # rows per h_c tensor
        self.neg_slope = neg_slope
        self.repeat_mode = "all"
        self.tail_skip = False

    def rows(self, b):
        return min(P, self.nloc - b * P)


FULL = Cfg(n=100000, e=1600000, d0=256, d1=128, d2=64)
assert FULL.gchunk == 25000 and FULL.lchunk == 3125


# --------------------------------------------------------------------------
# host-side preprocessing
# --------------------------------------------------------------------------

def prepare(cfg, x, edge_index, W1, b1, W2, b2):
    src = np.asarray(edge_index[0], dtype=np.int64)
    dst = np.asarray(edge_index[1], dtype=np.int64)
    NB, NK, NG, G = cfg.nblk, cfg.nchunk, cfg.ngrp, cfg.grp

    deg = (np.bincount(dst, minlength=cfg.n) + 1).astype(np.float32)
    dinv = (np.float32(1.0) / np.sqrt(deg)).astype(np.float32)

    core = dst // cfg.nloc
    dstl = dst - core * cfg.nloc
    b = dstl >> 7
    g = b // G
    csrc = src // cfg.nloc
    rsrc = src - csrc * cfg.nloc
    k = rsrc // cfg.lchunk
    cidx = csrc * cfg.lchunk + (rsrc - k * cfg.lchunk)

    key = ((core * NG + g) * NK + k) * NB + b
    order = np.argsort(key, kind="stable")
    cidx_s, b_s, k_s, g_s, core_s, dstl_s = (
        cidx[order], b[order], k[order], g[order], core[order], dstl[order])

    # per (core, g, k) counts -> shared tile counts T_gk = ceil(max/128)
    gk_id = (core_s * NG + g_s) * NK + k_s
    cnt_cgk = np.bincount(gk_id, minlength=cfg.ncores * NG * NK)
    cnt_cgk = cnt_cgk.reshape(cfg.ncores, NG, NK)
    Tgk = np.ceil(cnt_cgk.max(axis=0) / P).astype(np.int64)   # [NG, NK]

    # stream offsets: group-major, then chunk
    off_gk = np.zeros((NG, NK), np.int64)
    tot_tiles = 0
    for gg in range(NG):
        for kk in range(NK):
            off_gk[gg, kk] = tot_tiles * P
            tot_tiles += int(Tgk[gg, kk])
    etot = tot_tiles * P

    # per-(core,g,k) block counts for slot-span computation
    cgkb_id = ((core_s * NG + g_s) * NK + k_s) * NB + b_s
    cnt_cgkb = np.bincount(cgkb_id,
                           minlength=cfg.ncores * NG * NK * NB)
    cnt_cgkb = cnt_cgkb.reshape(cfg.ncores, NG, NK, NB)

    # static tile->slot spans: union over cores of the blocks a tile touches
    blocks_of_g = [list(range(gg * G, min((gg + 1) * G, NB)))
                   for gg in range(NG)]
    tile_slots = {}          # (g, k, t) -> list of slots
    for gg in range(NG):
        blks = blocks_of_g[gg]
        for kk in range(NK):
            t_gk = int(Tgk[gg, kk])
            if t_gk == 0:
                continue
            lo = np.full(t_gk, len(blks), np.int64)
            hi = np.full(t_gk, -1, np.int64)
            for c in range(cfg.ncores):
                e0 = 0
                for sl, bb in enumerate(blks):
                    cnt = int(cnt_cgkb[c, gg, kk, bb])
                    if cnt == 0:
                        continue
                    t0, t1 = e0 >> 7, (e0 + cnt - 1) >> 7
                    lo[t0:t1 + 1] = np.minimum(lo[t0:t1 + 1], sl)
                    hi[t0:t1 + 1] = np.maximum(hi[t0:t1 + 1], sl)
                    e0 += cnt
            for t in range(t_gk):
                if hi[t] >= 0:
                    tile_slots[(gg, kk, t)] = list(range(lo[t], hi[t] + 1))
                else:
                    tile_slots[(gg, kk, t)] = []

    xs = np.asarray(x, dtype=np.float32)
    W1 = np.asarray(W1, np.float32)
    W2 = np.asarray(W2, np.float32)
    b1bc = np.tile(np.asarray(b1, np.float32)[None, :], (P, 1))
    b2bc = np.tile(np.asarray(b2, np.float32)[None, :], (P, 1))

    in_maps = []
    for c in range(cfg.ncores):
        m = core_s == c
        cidx_c, g_c, k_c, dstl_c = cidx_s[m], g_s[m], k_s[m], dstl_s[m]
        gk_c = g_c * NK + k_c
        ne = len(cidx_c)
        if ne:
            newseg = np.r_[True, gk_c[1:] != gk_c[:-1]]
            seg_start = np.maximum.accumulate(
                np.where(newseg, np.arange(ne), 0))
            pos_in_seg = np.arange(ne) - seg_start
        else:
            pos_in_seg = np.zeros(0, np.int64)
        padded_pos = off_gk[g_c, k_c] + pos_in_seg

        idx_pad = np.full(etot, -1 if cfg.tail_skip else 0, np.int16)
        idx_pad[padded_pos] = cidx_c.astype(np.int16)
        dr_pad = np.full(etot, -1.0, np.float32)
        dr_pad[padded_pos] = (dstl_c - g_c * G * P).astype(np.float32)

        idx16 = idx_pad.reshape(etot // 16, 16).T.copy()
        idx_tab = np.tile(idx16, (8, 1))
        dstrel_tab = dr_pad.reshape(tot_tiles, P).T.copy()

        lo = c * cfg.nloc
        xT = np.ascontiguousarray(xs[lo:lo + cfg.nloc].T).astype(NP_BF16)
        dloc = dinv[lo:lo + cfg.nloc]
        dcols = np.ones(NB * P, np.float32)
        dcols[:cfg.nloc] = dloc
        dinv_cols = dcols.reshape(NB, P).T.copy()

        in_maps.append({
            "xT": xT, "dinv_cols": dinv_cols,
            "W1": W1.astype(NP_BF16), "W2": W2.astype(NP_BF16),
            "b1bc": b1bc, "b2bc": b2bc,
            "idx_tab": idx_tab, "dstrel_tab": dstrel_tab,
        })

    struct = dict(Tgk=Tgk, blocks_of_g=blocks_of_g, tot_tiles=tot_tiles,
                  tile_slots=tile_slots, off_gk=off_gk)
    return in_maps, struct


# --------------------------------------------------------------------------
# device program
# --------------------------------------------------------------------------

def build_program(cfg, struct, repeat=1):
    Tgk = struct["Tgk"]
    blocks_of_g = struct["blocks_of_g"]
    tot_tiles = struct["tot_tiles"]
    tile_slots = struct["tile_slots"]
    NB, NK, NG = cfg.nblk, cfg.nchunk, cfg.ngrp
    D0, D1, D2 = cfg.d0, cfg.d1, cfg.d2
    K0 = D0 // P
    G = cfg.grp

    # per-group tile layout: chunk-major columns
    grp_tiles = [int(Tgk[g].sum()) for g in range(NG)]
    grp_col0 = np.concatenate([[0], np.cumsum(grp_tiles)]).astype(int)
    gk_c0 = {}
    for g in range(NG):
        col = 0
        for k in range(NK):
            gk_c0[(g, k)] = col
            col += int(Tgk[g, k])
    TG = max(grp_tiles) if grp_tiles else 1

    # per (g, slot): ordered matmul contributions [(tilecol, slot)] for
    # start/stop flags
    slot_tiles = {}
    for g in range(NG):
        for k in range(NK):
            for t in range(int(Tgk[g, k])):
                for sl in tile_slots.get((g, k, t), []):
                    slot_tiles.setdefault((g, sl), []).append(
                        gk_c0[(g, k)] + t)

    nc = bacc.Bacc("TRN2", target_bir_lowering=False, debug=False,
                   num_devices=cfg.ncores, num_swdge_queues=4)

    xT = nc.dram_tensor("xT", [D0, cfg.nloc], BF16, kind="ExternalInput")
    dinv_t = nc.dram_tensor("dinv_cols", [P, NB], F32, kind="ExternalInput")
    W1 = nc.dram_tensor("W1", [D0, D1], BF16, kind="ExternalInput")
    W2 = nc.dram_tensor("W2", [D1, D2], BF16, kind="ExternalInput")
    b1t = nc.dram_tensor("b1bc", [P, D1], F32, kind="ExternalInput")
    b2t = nc.dram_tensor("b2bc", [P, D2], F32, kind="ExternalInput")
    idx_t = nc.dram_tensor("idx_tab", [P, tot_tiles * 8], I16,
                           kind="ExternalInput")
    drel_t = nc.dram_tensor("dstrel_tab", [P, tot_tiles], F32,
                            kind="ExternalInput")
    out_t = nc.dram_tensor("out_loc", [cfg.nloc, D2], F32,
                           kind="ExternalOutput")

    eq = mybir.AluOpType.is_equal
    mul = mybir.AluOpType.mult
    add = mybir.AluOpType.add
    lrelu = mybir.ActivationFunctionType.Lrelu
    copyf = mybir.ActivationFunctionType.Copy

    mode = getattr(cfg, "repeat_mode", "all")
    rg = [list(range(cfg.ncores))]

    with tile.TileContext(nc) as tc:
        with (
            tc.tile_pool(name="const", bufs=1) as cp,
            tc.tile_pool(name="dload", bufs=2) as dlp,
            tc.tile_pool(name="stg", bufs=2) as stp,
            tc.tile_pool(name="tbl", bufs=2) as tbp,
            tc.tile_pool(name="sm", bufs=6) as smp,
            tc.tile_pool(name="snk", bufs=3) as skp,
            tc.tile_pool(name="ost", bufs=3) as osp,
            tc.tile_pool(name="mm", bufs=4, space="PSUM") as mmp,
            tc.tile_pool(name="agg", bufs=2, space="PSUM") as aggp,
            tc.tile_pool(name="dram", bufs=1, space="DRAM") as drp,
        ):
            h1_loc = drp.tile([cfg.nloc, D1], BF16, tag="h1_loc")
            h2_loc = drp.tile([cfg.nloc, D1], BF16, tag="h2_loc")

            w1sb = cp.tile([P, K0, D1], BF16)
            for k0 in range(K0):
                nc.sync.dma_start(out=w1sb[:, k0, :],
                                  in_=W1[k0 * P:(k0 + 1) * P, :])
            w2sb = cp.tile([P, D2], BF16)
            nc.sync.dma_start(out=w2sb[:], in_=W2[:])
            b1sb = cp.tile([P, D1], F32)
            nc.sync.dma_start(out=b1sb[:], in_=b1t[:])
            b2sb = cp.tile([P, D2], F32)
            nc.sync.dma_start(out=b2sb[:], in_=b2t[:])
            dvsb = cp.tile([P, NB], F32)
            nc.sync.dma_start(out=dvsb[:], in_=dinv_t[:])
            iotg = cp.tile([P, G * P], F32)
            nc.gpsimd.iota(iotg[:], pattern=[[1, G * P]], base=0,
                           channel_multiplier=0,
                           allow_small_or_imprecise_dtypes=True)
            identf = cp.tile([P, P], F32)
            from concourse.masks import make_identity
            make_identity(nc, identf[:])

            # ------------- batched row-major DRAM <-> [P, j, D] SBUF moves
            def dma_rows(eng, dram, r0, nrow, sb, to_dram):
                jf = nrow // P
                if jf:
                    dap = dram[r0:r0 + jf * P, :].rearrange(
                        "(j p) d -> p j d", p=P)
                    if to_dram:
                        eng.dma_start(out=dap, in_=sb[:, :jf, :])
                    else:
                        eng.dma_start(out=sb[:, :jf, :], in_=dap)
                rem = nrow - jf * P
                if rem:
                    dap = dram[r0 + jf * P:r0 + nrow, :]
                    if to_dram:
                        eng.dma_start(out=dap, in_=sb[:rem, jf, :])
                    else:
                        eng.dma_start(out=sb[:rem, jf, :], in_=dap)

            # ------------- dense layer 1 (+ chunked AG1 interleaved)
            BB = 8
            NBB = math.ceil(NB / BB)

            def dense1(ag_after=None):
                for jb in range(NBB):
                    b0 = jb * BB
                    nbb = min(BB, NB - b0)
                    r0 = b0 * P
                    nrow = min(nbb * P, cfg.nloc - r0)
                    xt = dlp.tile([P, K0, BB * P], BF16, tag="xt")
                    for k0 in range(K0):
                        nc.sync.dma_start(
                            out=xt[:, k0, :nrow],
                            in_=xT[k0 * P:(k0 + 1) * P, r0:r0 + nrow])
                    hst = dlp.tile([P, BB, D1], BF16, tag="hst")
                    for j in range(nbb):
                        b = b0 + j
                        nb = cfg.rows(b)
                        ps = mmp.tile([P, D1], F32, tag="mm")
                        for k0 in range(K0):
                            nc.tensor.matmul(
                                out=ps[:nb, :],
                                lhsT=xt[:, k0, j * P:j * P + nb],
                                rhs=w1sb[:, k0, :],
                                start=(k0 == 0), stop=(k0 == K0 - 1))
                        nc.vector.tensor_scalar(
                            out=hst[:nb, j, :], in0=ps[:nb, :],
                            scalar1=dvsb[:nb, b:b + 1],
                            scalar2=None, op0=mul)
                    dma_rows(nc.scalar, h1_loc, r0, nrow, hst, True)
                    if ag_after is not None:
                        ag_after(jb)

            # ------------- aggregation (both layers share structure)
            first_groups = [True, True]

            def agg_phase(layer, hcs, h_self, sink, after_group=None):
                for g in range(NG):
                    Tg = grp_tiles[g]
                    if Tg == 0:
                        if after_group is not None:
                            after_group(g)
                        continue
                    gt0 = int(grp_col0[g])
                    stage_raw = stp.tile([P, TG * D1], BF16, tag="stage")
                    stage = stage_raw[:].rearrange("p (t d) -> p t d", d=D1)
                    if layer == 1 and g < 2 and first_groups[g]:
                        # stale-SBUF NaN guard for skipped gather tails
                        nc.vector.memset(stage_raw[:], 0.0)
                        first_groups[g] = False
                    idxt = tbp.tile([P, TG * 8], I16, tag="idx")
                    drt = tbp.tile([P, TG], F32, tag="drel")
                    nc.sync.dma_start(out=idxt[:, :Tg * 8],
                                      in_=idx_t[:, gt0 * 8:(gt0 + Tg) * 8])
                    nc.scalar.dma_start(out=drt[:, :Tg],
                                        in_=drel_t[:, gt0:gt0 + Tg])
                    blks = blocks_of_g[g]
                    selfb = skp.tile([P, G, D1], BF16, tag=f"self{layer}")
                    r0 = blks[0] * P
                    nrow = min(len(blks) * P, cfg.nloc - r0)
                    dma_rows(nc.scalar, h_self, r0, nrow, selfb, False)
                    for k in range(NK):
                        ntk = int(Tgk[g, k])
                        if ntk == 0:
                            continue
                        c0 = gk_c0[(g, k)]
                        nc.gpsimd.dma_gather(
                            out_ap=stage[:, c0:c0 + ntk, :],
                            in_ap=hcs[k][:],
                            idxs_ap=idxt[:, c0 * 8:(c0 + ntk) * 8],
                            num_idxs=ntk * P, num_idxs_reg=ntk * P,
                            elem_size=D1, single_packet=False,
                            queue_num=k)
                    agg = aggp.tile([P, G * D1], F32, tag="agg")
                    for slot, b in enumerate(blks):
                        tcols = slot_tiles.get((g, slot), [])
                        nb = cfg.rows(b)
                        nt = len(tcols)
                        for i, tcol in enumerate(tcols):
                            sm = smp.tile([P, P], BF16, tag="sm")
                            nc.vector.tensor_scalar(
                                out=sm[:], in0=iotg[:, slot * P:(slot + 1) * P],
                                scalar1=drt[:, tcol:tcol + 1],
                                scalar2=None, op0=eq)
                            nc.tensor.matmul(
                                out=agg[:, slot * D1:(slot + 1) * D1],
                                lhsT=sm[:],
                                rhs=stage[:, tcol, :],
                                start=(i == 0), stop=(i == nt - 1),
                                skip_group_check=True)
                        sink(g, slot, b, nb, nt,
                             agg[:, slot * D1:(slot + 1) * D1], selfb)
                    if after_group is not None:
                        after_group(g)

            # ------------- sinks
            h2st = [None]
            ost = [None]

            def sink1(g, slot, b, nb, ntot, aggsl, selfb):
                if slot == 0:
                    h2st[0] = osp.tile([P, G, D1], BF16, tag="h2st", name="h2st")
                t0 = skp.tile([P, D1], F32, tag="t0", bufs=4)
                if ntot:
                    nc.vector.tensor_tensor(out=t0[:nb, :], in0=aggsl[:nb, :],
                                            in1=selfb[:nb, slot, :], op=add)
                else:
                    nc.vector.tensor_copy(out=t0[:nb, :],
                                          in_=selfb[:nb, slot, :])
                t1 = skp.tile([P, D1], F32, tag="t1", bufs=4)
                nc.vector.tensor_scalar(out=t1[:nb, :], in0=t0[:nb, :],
                                        scalar1=dvsb[:nb, b:b + 1],
                                        scalar2=None, op0=mul)
                t2 = skp.tile([P, D1], F32, tag="t2", bufs=4)
                nc.vector.tensor_tensor(out=t2[:nb, :], in0=t1[:nb, :],
                                        in1=b1sb[:nb, :], op=add)
                nc.scalar.activation(out=h2st[0][:nb, slot, :], in_=t2[:nb, :],
                                     func=lrelu, bias=0.0,
                                     scale=dvsb[:nb, b:b + 1],
                                     alpha=float(cfg.neg_slope))
                if slot == len(blocks_of_g[g]) - 1:
                    r0 = blocks_of_g[g][0] * P
                    nrow = min(len(blocks_of_g[g]) * P, cfg.nloc - r0)
                    dma_rows(nc.sync, h2_loc, r0, nrow, h2st[0], True)

            def sink2(g, slot, b, nb, ntot, aggsl, selfb):
                if slot == 0:
                    ost[0] = osp.tile([P, G, D2], F32, tag="ost", name="ostt")
                t0 = skp.tile([P, D1], F32, tag="u0", bufs=4)
                if ntot:
                    nc.vector.tensor_tensor(out=t0[:nb, :], in0=aggsl[:nb, :],
                                            in1=selfb[:nb, slot, :], op=add)
                else:
                    nc.vector.tensor_copy(out=t0[:nb, :],
                                          in_=selfb[:nb, slot, :])
                t1 = skp.tile([P, D1], F32, tag="u1", bufs=4)
                nc.vector.tensor_scalar(out=t1[:nb, :], in0=t0[:nb, :],
                                        scalar1=dvsb[:nb, b:b + 1],
                                        scalar2=None, op0=mul)
                tp = mmp.tile([P, P], F32, tag="mm")
                nc.tensor.transpose(out=tp[:, :nb], in_=t1[:nb, :],
                                    identity=identf[:nb, :nb])
                t4 = skp.tile([P, P], BF16, tag="u4", bufs=4)
                nc.scalar.activation(out=t4[:, :nb], in_=tp[:, :nb],
                                     func=copyf, bias=0.0, scale=1.0)
                v = mmp.tile([P, D2], F32, tag="mm")
                nc.tensor.matmul(out=v[:nb, :], lhsT=t4[:, :nb],
                                 rhs=w2sb[:], start=True, stop=True)
                nc.vector.tensor_tensor(out=ost[0][:nb, slot, :],
                                        in0=v[:nb, :], in1=b2sb[:nb, :],
                                        op=add)
                if slot == len(blocks_of_g[g]) - 1:
                    r0 = blocks_of_g[g][0] * P
                    nrow = min(len(blocks_of_g[g]) * P, cfg.nloc - r0)
                    dma_rows(nc.sync, out_t, r0, nrow, ost[0], True)

            # ------------- program
            h1cs = h2cs = None
            for _rep in range(repeat):
                rep_all = mode == "all" or _rep == 0
                if rep_all or mode == "collectives":
                    h1cs = [drp.tile([cfg.gchunk, D1], BF16,
                                     tag=f"h1c{k}_{_rep}",
                                     addr_space="Shared",
                                     name=f"h1c{k}_{_rep}")
                            for k in range(NK)]
                    h2cs = [drp.tile([cfg.gchunk, D1], BF16,
                                     tag=f"h2c{k}_{_rep}",
                                     addr_space="Shared",
                                     name=f"h2c{k}_{_rep}")
                            for k in range(NK)]

                def ag(loc, dsts, k):
                    lo = k * cfg.lchunk
                    nc.gpsimd.collective_compute(
                        "AllGather", mybir.AluOpType.bypass,
                        replica_groups=rg,
                        ins=[loc[lo:lo + cfg.lchunk, :]],
                        outs=[dsts[k][:]])

                ag1_at = {}
                for k in range(NK):
                    need = (k + 1) * cfg.lchunk
                    ag1_at.setdefault(
                        min(math.ceil(need / (BB * P)) - 1, NBB - 1),
                        []).append(k)
                ag2_at = {}
                for k in range(NK):
                    need = (k + 1) * cfg.lchunk
                    ag2_at.setdefault(
                        min(math.ceil(need / (G * P)) - 1, NG - 1),
                        []).append(k)

                if rep_all:
                    if mode == "noov":
                        dense1()
                        for k in range(NK):
                            ag(h1_loc, h1cs, k)
                        agg_phase(1, h1cs, h1_loc, sink1)
                        for k in range(NK):
                            ag(h2_loc, h2cs, k)
                        agg_phase(2, h2cs, h2_loc, sink2)
                    else:
                        dense1(ag_after=lambda jb: [
                            ag(h1_loc, h1cs, k) for k in ag1_at.get(jb, [])])
                        agg_phase(1, h1cs, h1_loc, sink1,
                                  after_group=lambda g: [
                                      ag(h2_loc, h2cs, k)
                                      for k in ag2_at.get(g, [])])
                        agg_phase(2, h2cs, h2_loc, sink2)
                elif mode == "collectives":
                    for k in range(NK):
                        ag(h1_loc, h1cs, k)
                    for k in range(NK):
                        ag(h2_loc, h2cs, k)
                elif mode == "agg":
                    agg_phase(1, h1cs, h1_loc, sink1)
                    agg_phase(2, h2cs, h2_loc, sink2)
                elif mode == "a1":
                    agg_phase(1, h1cs, h1_loc, sink1)
                elif mode == "d1":
                    dense1()

    nc.compile()
    return nc


# --------------------------------------------------------------------------
# entry point
# --------------------------------------------------------------------------

_CACHE = {}


def _run(cfg, inputs):
    in_maps, struct = prepare(cfg, inputs["x"], inputs["edge_index"],
                              inputs["W1"], inputs["b1"],
                              inputs["W2"], inputs["b2"])
    key = (cfg.n, cfg.e, struct["Tgk"].tobytes())
    nc = _CACHE.get(key)
    if nc is None:
        nc = build_program(cfg, struct)
        _CACHE[key] = nc
    res = run_bass_kernel_spmd(nc, in_maps, list(range(cfg.ncores)))
    out = np.concatenate([res.results[c]["out_loc"]
                          for c in range(cfg.ncores)], axis=0)
    return out.astype(np.float32)


def kernel(x, edge_index, W1, b1, W2, b2):
    return _run(FULL, dict(x=x, edge_index=edge_index, W1=W1, b1=b1,
                           W2=W2, b2=b2))
